# revision 21
# baseline (speedup 1.0000x reference)
"""Trainium2 Bass kernel for batched tanh-query attention.

Per-batch computation (B=8, one batch per NeuronCore, pure data parallel):
    q = tanh(out_state)            [Q, H]    Q=K=2048, H=128
    S = q @ history.T              [Q, K]
    P = softmax(S, axis=K)
    attn = P @ history             [Q, H]

Flash-style, no HBM intermediates, computed in the transposed orientation
S_T[k, q] so the second matmul needs no transpose of P:
  MM1:  S_T[kb]   = ht[kb].T @ qT          (PE, fp32 PSUM, 512-wide chunks)
  exp:  E[kb]     = exp(S_T[kb])           (ACT, PSUM -> SBUF bf16)
  MM2:  attn_T   += hn[kb].T @ E[kb]       (PE, accumulate over kb)
  d:    pairwise-add tree over E[kb] (DVE bf16) then ones.T @ partials (PE)
  epilogue: PE-transpose attn_T and d back to q-major, multiply by 1/d,
  DMA out. Input operands are bf16; transposed layouts come from the DMA
  xbar transpose.
"""

import sys

for _p in ("/opt/trn_rl_repo", "/opt/trn_rl_repo/concourse"):
    if _p not in sys.path:
        sys.path.insert(0, _p)

import numpy as np

N_CORES = 8
SEQ = 2048
H = 128
P = 128
T = SEQ // P          # 16 seq tiles
NHALF = 2             # queries processed in 2 halves of 1024 (PSUM budget)
QH = SEQ // NHALF     # 1024
QTPH = QH // P        # 8 q-tiles per half
NC = 2                # 512-wide chunks per half
CW = QH // NC         # 512

_CACHE = {}


def _build():
    from concourse import bacc, bass, masks, mybir, tile

    f32 = mybir.dt.float32
    bf16 = mybir.dt.bfloat16
    AF = mybir.ActivationFunctionType

    nc = bacc.Bacc("TRN2", target_bir_lowering=False, debug=False,
                   num_devices=N_CORES)
    os_d = nc.dram_tensor("out_state", (SEQ, H), f32, kind="ExternalInput")
    h_d = nc.dram_tensor("history", (SEQ, H), f32, kind="ExternalInput")
    a_d = nc.dram_tensor("attn", (SEQ, H), f32, kind="ExternalOutput")

    with tile.TileContext(nc) as tc:
        with (
            tc.tile_pool(name="const", bufs=1) as constp,
            tc.tile_pool(name="big", bufs=1) as bigp,
            tc.tile_pool(name="stage", bufs=2) as stagep,
            tc.tile_pool(name="work", bufs=3) as workp,
            tc.tile_pool(name="expool", bufs=6) as expool,
            tc.tile_pool(name="dtree", bufs=4) as dtreep,
            tc.tile_pool(name="ps", bufs=4, space=bass.MemorySpace.PSUM) as psp,
            tc.tile_pool(name="psacc", bufs=1, space=bass.MemorySpace.PSUM) as pacc,
            tc.tile_pool(name="psd", bufs=2, space=bass.MemorySpace.PSUM) as psd,
        ):
            id_f32 = constp.tile([P, P], f32, tag="idf")
            masks.make_identity(nc, id_f32[:])
            id_bf = constp.tile([P, P], bf16, tag="idb")
            masks.make_identity(nc, id_bf[:])
            ones_bf = constp.tile([P, P], bf16, tag="ones")
            nc.vector.memset(ones_bf[:], 1.0)

            # persistent bf16 operands
            hn = bigp.tile([P, T, P], bf16, tag="hn")    # [k_in, t, h] natural
            ht = bigp.tile([P, T, P], bf16, tag="ht")    # [h, t, k_in] transposed
            qT = bigp.tile([P, T, P], bf16, tag="qT")    # [h, t, q_in] transposed

            # ---- load + preprocess (chunked so compute starts early) ----
            os_f = stagep.tile([P, T, H], f32, tag="ldin")
            hn_f = stagep.tile([P, T, H], f32, tag="ldin")
            os_v = os_d[:].rearrange("(t p) h -> p t h", p=P)
            hn_v = h_d[:].rearrange("(t p) h -> p t h", p=P)
            for j in range(4):
                sl = slice(4 * j, 4 * (j + 1))
                nc.gpsimd.dma_start(os_f[:, sl, :], os_v[:, sl, :])
                nc.gpsimd.dma_start(hn_f[:, sl, :], hn_v[:, sl, :])

            q_nat = stagep.tile([P, T, H], bf16, tag="qnat")
            for j in range(2):
                sl = slice(4 * j, 4 * (j + 1))
                nc.scalar.activation(q_nat[:, sl, :], os_f[:, sl, :], AF.Tanh)
                nc.vector.tensor_copy(hn[:, sl, :], hn_f[:, sl, :])

            # PE-transpose one [128,128] bf16 tile into a transposed layout
            def ptranspose(dst, src):
                tp = psp.tile([P, P], bf16, tag="st", name="tp")
                nc.tensor.transpose(tp[:], src, id_bf[:])
                nc.vector.tensor_copy(dst, tp[:])

            # upfront: the tiles the first loop iterations need
            for t in range(QTPH):
                ptranspose(qT[:, t, :], q_nat[:, t, :])
            for t in range(2):
                ptranspose(ht[:, t, :], hn[:, t, :])
            for j in range(2, 4):
                nc.scalar.activation(q_nat[:, 4 * j: 4 * (j + 1), :],
                                     os_f[:, 4 * j: 4 * (j + 1), :], AF.Tanh)
            # remaining transposes are interleaved into the half-0 loop below
            prefetch = [("h", t) for t in range(2, T)]
            prefetch += [("q", t) for t in range(QTPH, T)]

            # ---- epilogue helper: one output q-tile ----
            def emit_epi(qh, t, aT_sb, d_sb):
                dps = psp.tile([P, 1], f32, tag="st", name="dps")
                nc.tensor.transpose(dps[:], d_sb[0:1, P * t: P * (t + 1)],
                                    id_f32[0:1, 0:1])
                rc = workp.tile([P, 1], f32, tag="rc", name="rc")
                nc.vector.reciprocal(rc[:], dps[:])
                aps = psp.tile([P, P], f32, tag="st", name="aps")
                nc.tensor.transpose(aps[:], aT_sb[:, P * t: P * (t + 1)],
                                    id_f32[:])
                ot = workp.tile([P, P], f32, tag="ot", name="ot")
                nc.vector.tensor_scalar_mul(ot[:], aps[:], rc[:])
                row0 = qh * QH + P * t
                nc.sync.dma_start(a_d[row0: row0 + P, :], ot[:])

            epi_pending = []   # half-0 epilogue tiles, drained in half-1 loop

            # ---- main flash loop ----
            for qh in range(NHALF):
                attnT = pacc.tile([P, QH], f32, tag="acc")   # [h, q_local]
                # per-chunk d accumulators (PSUM)
                dbc = [psd.tile([P, CW], f32, tag="dbc", name=f"dbc{qh}_{i}")
                       for i in range(NC)]
                exprev = [None] * NC
                lvl1 = [[] for _ in range(NC)]
                for kb in range(T):
                    if qh == 0:
                        # interleave remaining input transposes: ht[kb] is
                        # consumed at iteration kb, prefetched 2 ahead
                        if kb == 2:
                            for j in range(2, 4):
                                sl = slice(4 * j, 4 * (j + 1))
                                nc.scalar.activation(q_nat[:, sl, :],
                                                     os_f[:, sl, :], AF.Tanh)
                                nc.vector.tensor_copy(hn[:, sl, :],
                                                      hn_f[:, sl, :])
                        for _ in range(2):
                            if prefetch:
                                kind, t = prefetch.pop(0)
                                src = hn if kind == "h" else q_nat
                                dst = ht if kind == "h" else qT
                                ptranspose(dst[:, t, :], src[:, t, :])
                    else:
                        # drain half-0's epilogue tiles
                        if epi_pending:
                            epi_pending.pop(0)()
                    first = kb == 0
                    last = kb == T - 1
                    for c in range(2):
                        st = psp.tile([P, CW], f32, tag="st")
                        rhs = qT[:, qh * QTPH + 4 * c: qh * QTPH + 4 * (c + 1), :]
                        nc.tensor.matmul(st[:], ht[:, kb, :], rhs,
                                         start=True, stop=True)
                        ex = expool.tile([P, CW], bf16, tag="ex")
                        nc.scalar.activation(ex[:], st[:], AF.Exp)
                        nc.tensor.matmul(attnT[:, CW * c: CW * (c + 1)],
                                         hn[:, kb, :], ex[:],
                                         start=first, stop=last)
                        # d: two levels of bf16 pair-adds on DVE, then
                        # accumulate quad sums via ones-matmul in PSUM
                        if kb % 2 == 0:
                            exprev[c] = ex
                        else:
                            t1 = dtreep.tile([P, CW], bf16, tag="l1")
                            nc.vector.tensor_add(t1[:], exprev[c][:], ex[:])
                            exprev[c] = None
                            lvl1[c].append(t1)
                            if len(lvl1[c]) == 2:
                                t2 = dtreep.tile([P, CW], bf16, tag="l2")
                                nc.vector.tensor_add(t2[:], lvl1[c][0][:],
                                                     lvl1[c][1][:])
                                lvl1[c] = []
                                nc.tensor.matmul(dbc[c][:], ones_bf[:], t2[:],
                                                 start=(kb == 3), stop=last)

                # ---- end of half: move accumulators to SBUF ----
                aT_sb = workp.tile([P, QH], f32, tag="atsb", name=f"aT{qh}")
                nc.vector.tensor_copy(aT_sb[:], attnT[:])
                d_sb = workp.tile([P, QH], f32, tag="dsb", name=f"d{qh}")
                for c in range(NC):
                    nc.vector.tensor_copy(d_sb[:, CW * c: CW * (c + 1)],
                                          dbc[c][:])
                if qh == 0:
                    epi_pending.extend(
                        (lambda t=t, a=aT_sb, d=d_sb: emit_epi(0, t, a, d))
                        for t in range(QTPH))
                else:
                    for t in range(QTPH):
                        emit_epi(1, t, aT_sb, d_sb)

    nc.compile()
    return nc


def _get_nc():
    if "nc" not in _CACHE:
        _CACHE["nc"] = _build()
    return _CACHE["nc"]


def _run(out_state, history, trace=False):
    from concourse.bass_utils import run_bass_kernel_spmd

    nc = _get_nc()
    out_state = np.ascontiguousarray(out_state, dtype=np.float32)
    history = np.ascontiguousarray(history, dtype=np.float32)
    in_maps = [
        {"out_state": out_state[b], "history": history[b]}
        for b in range(N_CORES)
    ]
    res = run_bass_kernel_spmd(nc, in_maps, core_ids=list(range(N_CORES)),
                               trace=trace)
    attn = np.stack([res.results[b]["attn"] for b in range(N_CORES)], axis=0)
    return attn.astype(np.float32), res


def kernel(out_state, history):
    attn, _ = _run(out_state, history)
    return attn


# revision 22
# speedup vs baseline: 1.0203x; 1.0203x over previous
"""Trainium2 Bass kernel for batched tanh-query attention.

Per-batch computation (B=8, one batch per NeuronCore, pure data parallel):
    q = tanh(out_state)            [Q, H]    Q=K=2048, H=128
    S = q @ history.T              [Q, K]
    P = softmax(S, axis=K)
    attn = P @ history             [Q, H]

Flash-style, no HBM intermediates, computed in the transposed orientation
S_T[k, q] so the second matmul needs no transpose of P:
  MM1:  S_T[kb]   = ht[kb].T @ qT          (PE, fp32 PSUM, 512-wide chunks)
  exp:  E[kb]     = exp(S_T[kb])           (ACT, PSUM -> SBUF bf16)
  MM2:  attn_T   += hn[kb].T @ E[kb]       (PE, accumulate over kb)
  d:    pairwise-add tree over E[kb] (DVE bf16) then ones.T @ partials (PE)
  epilogue: PE-transpose attn_T and d back to q-major, multiply by 1/d,
  DMA out. Input operands are bf16; transposed layouts come from the DMA
  xbar transpose.
"""

import sys

for _p in ("/opt/trn_rl_repo", "/opt/trn_rl_repo/concourse"):
    if _p not in sys.path:
        sys.path.insert(0, _p)

import numpy as np

N_CORES = 8
SEQ = 2048
H = 128
P = 128
T = SEQ // P          # 16 seq tiles
NHALF = 2             # queries processed in 2 halves of 1024 (PSUM budget)
QH = SEQ // NHALF     # 1024
QTPH = QH // P        # 8 q-tiles per half
NC = 2                # 512-wide chunks per half
CW = QH // NC         # 512

_CACHE = {}


def _build():
    from concourse import bacc, bass, masks, mybir, tile

    f32 = mybir.dt.float32
    bf16 = mybir.dt.bfloat16
    AF = mybir.ActivationFunctionType

    nc = bacc.Bacc("TRN2", target_bir_lowering=False, debug=False,
                   num_devices=N_CORES)
    os_d = nc.dram_tensor("out_state", (SEQ, H), f32, kind="ExternalInput")
    h_d = nc.dram_tensor("history", (SEQ, H), f32, kind="ExternalInput")
    a_d = nc.dram_tensor("attn", (SEQ, H), f32, kind="ExternalOutput")

    with tile.TileContext(nc) as tc:
        with (
            tc.tile_pool(name="const", bufs=1) as constp,
            tc.tile_pool(name="big", bufs=1) as bigp,
            tc.tile_pool(name="stage", bufs=2) as stagep,
            tc.tile_pool(name="work", bufs=3) as workp,
            tc.tile_pool(name="expool", bufs=6) as expool,
            tc.tile_pool(name="dtree", bufs=4) as dtreep,
            tc.tile_pool(name="ps", bufs=4, space=bass.MemorySpace.PSUM) as psp,
            tc.tile_pool(name="psacc", bufs=1, space=bass.MemorySpace.PSUM) as pacc,
            tc.tile_pool(name="psd", bufs=2, space=bass.MemorySpace.PSUM) as psd,
        ):
            id_f32 = constp.tile([P, P], f32, tag="idf")
            masks.make_identity(nc, id_f32[:])
            id_bf = constp.tile([P, P], bf16, tag="idb")
            masks.make_identity(nc, id_bf[:])
            ones_bf = constp.tile([P, P], bf16, tag="ones")
            nc.vector.memset(ones_bf[:], 1.0)

            # persistent bf16 operands
            hn = bigp.tile([P, T, P], bf16, tag="hn")    # [k_in, t, h] natural
            ht = bigp.tile([P, T, P], bf16, tag="ht")    # [h, t, k_in] transposed
            qT = bigp.tile([P, T, P], bf16, tag="qT")    # [h, t, q_in] transposed

            # ---- load + preprocess (chunked so compute starts early) ----
            os_f = stagep.tile([P, T, H], f32, tag="ldin")
            hn_f = stagep.tile([P, T, H], f32, tag="ldin")
            os_v = os_d[:].rearrange("(t p) h -> p t h", p=P)
            hn_v = h_d[:].rearrange("(t p) h -> p t h", p=P)
            for j in range(4):
                sl = slice(4 * j, 4 * (j + 1))
                nc.sync.dma_start(os_f[:, sl, :], os_v[:, sl, :])
                nc.sync.dma_start(hn_f[:, sl, :], hn_v[:, sl, :])

            q_nat = stagep.tile([P, T, H], bf16, tag="qnat")
            for j in range(2):
                sl = slice(4 * j, 4 * (j + 1))
                nc.scalar.activation(q_nat[:, sl, :], os_f[:, sl, :], AF.Tanh)
                nc.vector.tensor_copy(hn[:, sl, :], hn_f[:, sl, :])

            # PE-transpose one [128,128] bf16 tile into a transposed layout
            def ptranspose(dst, src):
                tp = psp.tile([P, P], bf16, tag="st", name="tp")
                nc.tensor.transpose(tp[:], src, id_bf[:])
                nc.vector.tensor_copy(dst, tp[:])

            # upfront: the tiles the first loop iterations need
            for t in range(QTPH):
                ptranspose(qT[:, t, :], q_nat[:, t, :])
            for t in range(2):
                ptranspose(ht[:, t, :], hn[:, t, :])
            for j in range(2, 4):
                nc.scalar.activation(q_nat[:, 4 * j: 4 * (j + 1), :],
                                     os_f[:, 4 * j: 4 * (j + 1), :], AF.Tanh)
            # remaining transposes are interleaved into the half-0 loop below
            prefetch = [("h", t) for t in range(2, T)]
            prefetch += [("q", t) for t in range(QTPH, T)]

            # ---- epilogue helper: one output q-tile ----
            def emit_epi(qh, t, aT_sb, d_sb):
                dps = psp.tile([P, 1], f32, tag="st", name="dps")
                nc.tensor.transpose(dps[:], d_sb[0:1, P * t: P * (t + 1)],
                                    id_f32[0:1, 0:1])
                rc = workp.tile([P, 1], f32, tag="rc", name="rc")
                nc.vector.reciprocal(rc[:], dps[:])
                aps = psp.tile([P, P], f32, tag="st", name="aps")
                nc.tensor.transpose(aps[:], aT_sb[:, P * t: P * (t + 1)],
                                    id_f32[:])
                ot = workp.tile([P, P], f32, tag="ot", name="ot")
                nc.vector.tensor_scalar_mul(ot[:], aps[:], rc[:])
                row0 = qh * QH + P * t
                nc.sync.dma_start(a_d[row0: row0 + P, :], ot[:])

            epi_pending = []   # half-0 epilogue tiles, drained in half-1 loop

            # ---- main flash loop ----
            for qh in range(NHALF):
                attnT = pacc.tile([P, QH], f32, tag="acc")   # [h, q_local]
                # per-chunk d accumulators (PSUM)
                dbc = [psd.tile([P, CW], f32, tag="dbc", name=f"dbc{qh}_{i}")
                       for i in range(NC)]
                exprev = [None] * NC
                lvl1 = [[] for _ in range(NC)]
                for kb in range(T):
                    if qh == 0:
                        # interleave remaining input transposes: ht[kb] is
                        # consumed at iteration kb, prefetched 2 ahead
                        if kb == 2:
                            for j in range(2, 4):
                                sl = slice(4 * j, 4 * (j + 1))
                                nc.scalar.activation(q_nat[:, sl, :],
                                                     os_f[:, sl, :], AF.Tanh)
                                nc.vector.tensor_copy(hn[:, sl, :],
                                                      hn_f[:, sl, :])
                        for _ in range(2):
                            if prefetch:
                                kind, t = prefetch.pop(0)
                                src = hn if kind == "h" else q_nat
                                dst = ht if kind == "h" else qT
                                ptranspose(dst[:, t, :], src[:, t, :])
                    else:
                        # drain half-0's epilogue tiles
                        if epi_pending:
                            epi_pending.pop(0)()
                    first = kb == 0
                    last = kb == T - 1
                    for c in range(2):
                        st = psp.tile([P, CW], f32, tag="st")
                        rhs = qT[:, qh * QTPH + 4 * c: qh * QTPH + 4 * (c + 1), :]
                        nc.tensor.matmul(st[:], ht[:, kb, :], rhs,
                                         start=True, stop=True)
                        ex = expool.tile([P, CW], bf16, tag="ex")
                        nc.scalar.activation(ex[:], st[:], AF.Exp)
                        nc.tensor.matmul(attnT[:, CW * c: CW * (c + 1)],
                                         hn[:, kb, :], ex[:],
                                         start=first, stop=last)
                        # d: two levels of bf16 pair-adds on DVE, then
                        # accumulate quad sums via ones-matmul in PSUM
                        if kb % 2 == 0:
                            exprev[c] = ex
                        else:
                            t1 = dtreep.tile([P, CW], bf16, tag="l1")
                            nc.vector.tensor_add(t1[:], exprev[c][:], ex[:])
                            exprev[c] = None
                            lvl1[c].append(t1)
                            if len(lvl1[c]) == 2:
                                t2 = dtreep.tile([P, CW], bf16, tag="l2")
                                nc.vector.tensor_add(t2[:], lvl1[c][0][:],
                                                     lvl1[c][1][:])
                                lvl1[c] = []
                                nc.tensor.matmul(dbc[c][:], ones_bf[:], t2[:],
                                                 start=(kb == 3), stop=last)

                # ---- end of half: move accumulators to SBUF ----
                aT_sb = workp.tile([P, QH], f32, tag="atsb", name=f"aT{qh}")
                nc.vector.tensor_copy(aT_sb[:], attnT[:])
                d_sb = workp.tile([P, QH], f32, tag="dsb", name=f"d{qh}")
                for c in range(NC):
                    nc.vector.tensor_copy(d_sb[:, CW * c: CW * (c + 1)],
                                          dbc[c][:])
                if qh == 0:
                    epi_pending.extend(
                        (lambda t=t, a=aT_sb, d=d_sb: emit_epi(0, t, a, d))
                        for t in range(QTPH))
                else:
                    for t in range(QTPH):
                        emit_epi(1, t, aT_sb, d_sb)

    nc.compile()
    return nc


def _get_nc():
    if "nc" not in _CACHE:
        _CACHE["nc"] = _build()
    return _CACHE["nc"]


def _run(out_state, history, trace=False):
    from concourse.bass_utils import run_bass_kernel_spmd

    nc = _get_nc()
    out_state = np.ascontiguousarray(out_state, dtype=np.float32)
    history = np.ascontiguousarray(history, dtype=np.float32)
    in_maps = [
        {"out_state": out_state[b], "history": history[b]}
        for b in range(N_CORES)
    ]
    res = run_bass_kernel_spmd(nc, in_maps, core_ids=list(range(N_CORES)),
                               trace=trace)
    attn = np.stack([res.results[b]["attn"] for b in range(N_CORES)], axis=0)
    return attn.astype(np.float32), res


def kernel(out_state, history):
    attn, _ = _run(out_state, history)
    return attn


# revision 25
# speedup vs baseline: 1.1866x; 1.1630x over previous
"""Trainium2 Bass kernel for batched tanh-query attention.

Per-batch computation (B=8, one batch per NeuronCore, pure data parallel):
    q = tanh(out_state)            [Q, H]    Q=K=2048, H=128
    S = q @ history.T              [Q, K]
    P = softmax(S, axis=K)
    attn = P @ history             [Q, H]

Flash-style, no HBM intermediates, computed in the transposed orientation
S_T[k, q] so the second matmul needs no transpose of P. Queries are
processed in 4 quarters of 512 columns; each quarter runs two software
phases that overlap across quarters:
  A(q): per kb-pair  S_T = ht[kb].T @ qT  (PE) -> exp FD=1024 (ACT, bf16)
        + two levels of bf16 pair-adds on DVE for the softmax denominator
  B(q): 16 accumulating  attn_T += hn[kb].T @ expS  matmuls + 4 ones-matmuls
        for d, emitted in two dense batches inside A(q+1) so PE runs them
        back-to-back while ACT keeps computing exps.
Epilogue per quarter (PE-transpose attn_T / d back to q-major, 1/d scale,
DMA out) is drained into later quarters' A phases.
"""

import sys

for _p in ("/opt/trn_rl_repo", "/opt/trn_rl_repo/concourse"):
    if _p not in sys.path:
        sys.path.insert(0, _p)

import numpy as np

N_CORES = 8
SEQ = 2048
H = 128
P = 128
T = SEQ // P          # 16 seq tiles
NQ = 4                # query quarters
QW = SEQ // NQ        # 512
QTPQ = QW // P        # 4 q-tiles per quarter
NPAIR = T // 2        # 8 kb-pairs per quarter

_CACHE = {}


def _build():
    from concourse import bacc, bass, masks, mybir, tile

    f32 = mybir.dt.float32
    bf16 = mybir.dt.bfloat16
    AF = mybir.ActivationFunctionType

    nc = bacc.Bacc("TRN2", target_bir_lowering=False, debug=False,
                   num_devices=N_CORES)
    os_d = nc.dram_tensor("out_state", (SEQ, H), f32, kind="ExternalInput")
    h_d = nc.dram_tensor("history", (SEQ, H), f32, kind="ExternalInput")
    a_d = nc.dram_tensor("attn", (SEQ, H), f32, kind="ExternalOutput")

    with tile.TileContext(nc) as tc:
        with (
            tc.tile_pool(name="const", bufs=1) as constp,
            tc.tile_pool(name="big", bufs=1) as bigp,
            tc.tile_pool(name="stage", bufs=2) as stagep,
            tc.tile_pool(name="work", bufs=4) as workp,
            tc.tile_pool(name="expool", bufs=11) as expool,
            tc.tile_pool(name="dtree", bufs=6) as dtreep,
            tc.tile_pool(name="ps", bufs=2, space=bass.MemorySpace.PSUM) as psp,
            tc.tile_pool(name="psacc", bufs=2, space=bass.MemorySpace.PSUM) as pacc,
            tc.tile_pool(name="psd", bufs=2, space=bass.MemorySpace.PSUM) as psd,
        ):
            id_f32 = constp.tile([P, P], f32, tag="idf")
            masks.make_identity(nc, id_f32[:])
            id_bf = constp.tile([P, P], bf16, tag="idb")
            masks.make_identity(nc, id_bf[:])
            ones_bf = constp.tile([P, P], bf16, tag="ones")
            nc.vector.memset(ones_bf[:], 1.0)

            # persistent bf16 operands
            hn = bigp.tile([P, T, P], bf16, tag="hn")    # [k_in, t, h] natural
            ht = bigp.tile([P, T, P], bf16, tag="ht")    # [h, t, k_in]
            qT = bigp.tile([P, T, P], bf16, tag="qT")    # [h, t, q_in]

            # ---- load + preprocess (chunked so compute starts early) ----
            os_f = stagep.tile([P, T, H], f32, tag="ldin")
            hn_f = stagep.tile([P, T, H], f32, tag="ldin")
            os_v = os_d[:].rearrange("(t p) h -> p t h", p=P)
            hn_v = h_d[:].rearrange("(t p) h -> p t h", p=P)
            for j in range(4):
                sl = slice(4 * j, 4 * (j + 1))
                nc.sync.dma_start(os_f[:, sl, :], os_v[:, sl, :])
                nc.sync.dma_start(hn_f[:, sl, :], hn_v[:, sl, :])

            q_nat = stagep.tile([P, T, H], bf16, tag="qnat")
            for j in range(2):
                sl = slice(4 * j, 4 * (j + 1))
                nc.scalar.activation(q_nat[:, sl, :], os_f[:, sl, :], AF.Tanh)
                nc.vector.tensor_copy(hn[:, sl, :], hn_f[:, sl, :])

            def late_prep():
                for j in range(2, 4):
                    sl = slice(4 * j, 4 * (j + 1))
                    nc.scalar.activation(q_nat[:, sl, :], os_f[:, sl, :],
                                         AF.Tanh)
                    nc.vector.tensor_copy(hn[:, sl, :], hn_f[:, sl, :])

            # PE-transpose one [128,128] bf16 tile into a transposed layout
            def ptranspose(dst, src):
                tp = psd.tile([P, P], bf16, tag="dbc", name="tp")
                nc.tensor.transpose(tp[:], src, id_bf[:])
                nc.vector.tensor_copy(dst, tp[:])

            # aux work queue: input transposes now, epilogue tiles later
            aux = []

            def drain_aux(n):
                for _ in range(n):
                    if aux:
                        aux.pop(0)()

            def tp_job(kind, t):
                def job():
                    src = hn if kind == "h" else q_nat
                    dst = ht if kind == "h" else qT
                    ptranspose(dst[:, t, :], src[:, t, :])
                return job

            # upfront: tiles the first A-phase pairs need
            for t in range(QTPQ):
                ptranspose(qT[:, t, :], q_nat[:, t, :])
            for t in range(2):
                ptranspose(ht[:, t, :], hn[:, t, :])
            aux.extend(tp_job("h", t) for t in range(2, T))
            aux.extend(tp_job("q", t) for t in range(QTPQ, T))

            # ---- epilogue helper: one output q-tile of 128 rows ----
            def emit_epi(q, t, aT_sb, d_sb):
                dps = pacc.tile([P, 1], f32, tag="acc", name="dps")
                nc.tensor.transpose(dps[:], d_sb[0:1, P * t: P * (t + 1)],
                                    id_f32[0:1, 0:1])
                rc = workp.tile([P, 1], f32, tag="rc", name="rc")
                nc.vector.reciprocal(rc[:], dps[:])
                aps = pacc.tile([P, P], f32, tag="acc", name="aps")
                nc.tensor.transpose(aps[:], aT_sb[:, P * t: P * (t + 1)],
                                    id_f32[:])
                ot = workp.tile([P, P], f32, tag="ot", name="ot")
                nc.vector.tensor_scalar_mul(ot[:], aps[:], rc[:])
                row0 = q * QW + P * t
                nc.sync.dma_start(a_d[row0: row0 + P, :], ot[:])

            # ---- build per-quarter phase closures ----
            ex_tiles = [[] for _ in range(NQ)]
            lvl2s = [[] for _ in range(NQ)]
            accs = [None] * NQ
            dqs = [None] * NQ
            l1prev = [None] * NQ

            def emit_pair(q, p):
                if accs[q] is None:
                    accs[q] = pacc.tile([P, QW], f32, tag="acc",
                                        name=f"acc{q}")
                    dqs[q] = psd.tile([P, QW], f32, tag="dbc", name=f"dq{q}")
                kb0 = 2 * p
                st = psp.tile([P, 2 * QW], f32, tag="st", name="st")
                rhs = qT[:, QTPQ * q: QTPQ * (q + 1), :]
                nc.tensor.matmul(st[:, 0:QW], ht[:, kb0, :], rhs,
                                 start=True, stop=True)
                nc.tensor.matmul(st[:, QW:], ht[:, kb0 + 1, :], rhs,
                                 start=True, stop=True)
                ex = expool.tile([P, 2 * QW], bf16, tag="ex", name="ex")
                nc.scalar.activation(ex[:], st[:], AF.Exp)
                ex_tiles[q].append(ex)
                # d: in-tile pair add, then quad add (DVE, bf16)
                t1 = dtreep.tile([P, QW], bf16, tag="l1", name="t1")
                nc.vector.tensor_add(t1[:], ex[:, 0:QW], ex[:, QW:])
                if l1prev[q] is None:
                    l1prev[q] = t1
                else:
                    t2 = dtreep.tile([P, QW], bf16, tag="l2", name="t2")
                    nc.vector.tensor_add(t2[:], l1prev[q][:], t1[:])
                    l1prev[q] = None
                    lvl2s[q].append(t2)

            def emit_B(q, half):
                kbs = range(8 * half, 8 * (half + 1))
                for kb in kbs:
                    nc.tensor.matmul(
                        accs[q][:], hn[:, kb, :],
                        ex_tiles[q][kb // 2][:, QW * (kb % 2): QW * (kb % 2 + 1)],
                        start=(kb == 0), stop=(kb == T - 1))
                if half == 1:
                    for i, t2 in enumerate(lvl2s[q]):
                        nc.tensor.matmul(dqs[q][:], ones_bf[:], t2[:],
                                         start=(i == 0), stop=(i == 3))
                    # move accumulators to SBUF, queue epilogue tiles
                    aT_sb = workp.tile([P, QW], f32, tag="atsb",
                                       name=f"aT{q}")
                    nc.vector.tensor_copy(aT_sb[:], accs[q][:])
                    d_sb = workp.tile([P, QW], f32, tag="dsb", name=f"d{q}")
                    nc.vector.tensor_copy(d_sb[:], dqs[q][:])
                    aux.extend(
                        (lambda t=t, a=aT_sb, d=d_sb, q=q: emit_epi(q, t, a, d))
                        for t in range(QTPQ))

            # ---- emission schedule ----
            for q in range(NQ):
                for p in range(NPAIR):
                    if q == 0 and p >= 1:
                        # ht transposes, one pair ahead of their consumer
                        drain_aux(2)
                    emit_pair(q, p)
                    if q == 0 and p == 2:
                        late_prep()
                    if q > 0 and p == 1:
                        emit_B(q - 1, 0)
                    if q > 0 and p == 3:
                        emit_B(q - 1, 1)
                    if p >= 4:
                        drain_aux(2)
            emit_B(NQ - 1, 0)
            emit_B(NQ - 1, 1)
            while aux:
                aux.pop(0)()

    nc.compile()
    return nc


def _get_nc():
    if "nc" not in _CACHE:
        _CACHE["nc"] = _build()
    return _CACHE["nc"]


def _run(out_state, history, trace=False):
    from concourse.bass_utils import run_bass_kernel_spmd

    nc = _get_nc()
    out_state = np.ascontiguousarray(out_state, dtype=np.float32)
    history = np.ascontiguousarray(history, dtype=np.float32)
    in_maps = [
        {"out_state": out_state[b], "history": history[b]}
        for b in range(N_CORES)
    ]
    res = run_bass_kernel_spmd(nc, in_maps, core_ids=list(range(N_CORES)),
                               trace=trace)
    attn = np.stack([res.results[b]["attn"] for b in range(N_CORES)], axis=0)
    return attn.astype(np.float32), res


def kernel(out_state, history):
    attn, _ = _run(out_state, history)
    return attn
